# revision 58
# baseline (speedup 1.0000x reference)
"""Trainium2 Bass kernel for multi-head causal self-attention.

Tensor-parallel over 8 NeuronCores: each core owns 2 of the 16 heads.
Per core (SPMD, identical program, different weight shards), fused
chunk pipeline per batch: for each 512-row s-chunk, QKV projections
for that chunk, then causal attention for that q-chunk over all key
blocks produced so far.

All matmuls run in bf16 (fp32 PSUM accumulation). Scores kept
transposed [k, q]; softmax denominator via a ones column appended to
each head's V block ([V_h | 1], 129-wide PV matmuls). The exp on the
scalar engine is the attention inner-loop bottleneck, so the
transpose + output-projection work of q-chunk qi is deferred and
injected into qi+1's score/PV loop to fill the tensor-engine bubbles.

Both heads share each accumulator PSUM bank ([128,260]: h0 at col 0,
h1 at col 130): the first matmul of h0 clears the bank's has_written
bits, h1's first matmul uses start=False and overwrites via the
per-element has_written semantics.

Host: shards weights, pre-transposes X to bf16, sums 8 bf16 partials,
folds in bo + bv @ Wo (bv commutes through softmax; bk cancels).
"""
import numpy as np
from contextlib import ExitStack

import concourse.bass as bass
import concourse.tile as tile
from concourse import bacc, mybir
from concourse.bass_utils import run_bass_kernel_spmd

# Problem shape (hardcoded per contract)
B, S, D = 2, 2048, 2048
H, DH = 16, 128
N_CORES = 8
HL = H // N_CORES          # heads per core = 2
DHL = HL * DH              # 256
SC = 512                   # s-chunk width
NSC = S // SC              # 4 chunks per batch
NKB = S // 128             # 16 key blocks per batch
NDC = D // 128             # 16 contraction blocks

F32 = mybir.dt.float32
BF16 = mybir.dt.bfloat16
AF = mybir.ActivationFunctionType

_cached_nc = None


def _mm(nc, out, lhsT, rhs, start, stop, skip_group_check=False):
    nc.tensor.matmul(out, lhsT, rhs, start=start, stop=stop,
                     skip_group_check=skip_group_check)


def build_nc():
    nc = bacc.Bacc("TRN2", target_bir_lowering=False, debug=False,
                   num_devices=N_CORES)

    # all inputs pre-laid-out host-side as [128 partitions, ...contiguous]
    # so every DMA moves 4-8KB contiguous rows per partition (descriptor-
    # rate-bound otherwise: 1KB rows measured ~45 GB/s per queue)
    xt4 = nc.dram_tensor("xt4", [B, NSC, 4, 128, 4, SC], BF16,
                         kind="ExternalInput").ap()
    wq = nc.dram_tensor("wq", [128, NDC, DHL], BF16, kind="ExternalInput").ap()
    wk = nc.dram_tensor("wk", [128, NDC, DHL], BF16, kind="ExternalInput").ap()
    wv = nc.dram_tensor("wv", [128, NDC, DHL], BF16, kind="ExternalInput").ap()
    bqt_d = nc.dram_tensor("bqt", [128, 16], F32, kind="ExternalInput").ap()
    wo = nc.dram_tensor("wo", [HL, 128, D], BF16, kind="ExternalInput").ap()
    mi_d = nc.dram_tensor("mi", [128, 256], BF16, kind="ExternalInput").ap()
    out = nc.dram_tensor("out", [B, S, D], BF16, kind="ExternalOutput").ap()

    with tile.TileContext(nc) as tc, ExitStack() as ctx:
        pp = ctx.enter_context(tc.tile_pool(name="persist", bufs=1))

        wq_t = pp.tile([128, NDC, DHL], BF16)
        wk_t = pp.tile([128, NDC, DHL], BF16)
        wv_t = pp.tile([128, NDC, DHL], BF16)
        wo_t = pp.tile([128, HL, D], BF16)
        bqt = pp.tile([128, 16], F32)
        mi = pp.tile([128, 256], BF16)
        mask = mi[:, 0:128]
        ident = mi[:, 128:256]

        # need-ordered startup on the two hardware DMA queues (sync/scalar);
        # gpsimd's software queue has ~5us pickup latency, use it mid-kernel

        # double-buffered across batches
        qt_b = [pp.tile([128, HL, S], BF16, name=f"qt{i}") for i in range(2)]
        kt_b = [pp.tile([128, HL, S], BF16, name=f"kt{i}") for i in range(2)]
        vcat_b = [pp.tile([128, NKB, HL, 129], BF16, name=f"vc{i}")
                  for i in range(2)]

        xp = ctx.enter_context(tc.tile_pool(name="xtp", bufs=3))
        qkvp = ctx.enter_context(
            tc.tile_pool(name="qkvp", bufs=3, space="PSUM"))
        scp = ctx.enter_context(tc.tile_pool(name="scp", bufs=2, space="PSUM"))
        accp = ctx.enter_context(tc.tile_pool(name="accp", bufs=1, space="PSUM"))
        exp_ = ctx.enter_context(tc.tile_pool(name="exp", bufs=10))
        sm = ctx.enter_context(tc.tile_pool(name="sm", bufs=2))

        deferred = []  # closures: transpose + outproj work of previous qi
        cast_split = [False]  # route half the ot casts to scalar when set

        for b in range(B):
            qt, kt, vcat = qt_b[b % 2], kt_b[b % 2], vcat_b[b % 2]
            nc.gpsimd.memset(vcat[:, :, :, 128:129], 1.0)

            for sc in range(NSC):
                # ---- QKV projections for this chunk ----
                xt_t = xp.tile([128, NDC, SC], BF16, tag="xt")
                if b == 0 and sc == 0:
                    # startup burst: fine slices interleaved by need-time
                    # across both hw queues (aggregate DMA ~370GB/s shared)
                    nc.scalar.dma_start(out=wq_t[:, 0:4, :], in_=wq[:, 0:4, :])
                    nc.sync.dma_start(out=xt_t[:, 0:4, :], in_=xt4[0, 0, 0])
                    nc.scalar.dma_start(out=wq_t[:, 4:8, :], in_=wq[:, 4:8, :])
                    nc.sync.dma_start(out=xt_t[:, 4:8, :], in_=xt4[0, 0, 1])
                    nc.scalar.dma_start(out=xt_t[:, 8:12, :],
                                        in_=xt4[0, 0, 2])
                    nc.sync.dma_start(out=wq_t[:, 8:NDC, :],
                                      in_=wq[:, 8:NDC, :])
                    nc.scalar.dma_start(out=xt_t[:, 12:16, :],
                                        in_=xt4[0, 0, 3])
                    nc.sync.dma_start(out=bqt, in_=bqt_d)
                    nc.sync.dma_start(out=wk_t[:, 0:8, :], in_=wk[:, 0:8, :])
                    nc.scalar.dma_start(out=wk_t[:, 8:NDC, :],
                                        in_=wk[:, 8:NDC, :])
                    nc.sync.dma_start(out=wv_t, in_=wv)
                    nc.scalar.dma_start(out=mi, in_=mi_d)
                elif b == 0 and sc == 1:
                    # scalar engine reaches these after chunk0's activations,
                    # keeping the startup burst clear of non-critical bytes
                    for part in range(4):
                        nc.scalar.dma_start(
                            out=xt_t[:, 4 * part:4 * part + 4, :],
                            in_=xt4[b, sc, part])
                    nc.scalar.dma_start(out=wo_t[:, 0, :], in_=wo[0])
                    nc.scalar.dma_start(out=wo_t[:, 1, :], in_=wo[1])
                else:
                    for part in range(4):
                        eng = nc.gpsimd if part % 2 == 0 else nc.sync
                        eng.dma_start(
                            out=xt_t[:, 4 * part:4 * part + 4, :],
                            in_=xt4[b, sc, part])
                for h in range(HL):
                    psq = qkvp.tile([128, SC], F32, tag="qkv")
                    for dc in range(NDC):
                        _mm(nc, psq, wq_t[:, dc, h * 128:(h + 1) * 128],
                            xt_t[:, dc, :], dc == 0, dc == NDC - 1)
                    nc.scalar.activation(
                        out=qt[:, h, sc * SC:(sc + 1) * SC], in_=psq,
                        func=AF.Identity, bias=bqt[:, h:h + 1], scale=1.0)
                    psk = qkvp.tile([128, SC], F32, tag="qkv")
                    for dc in range(NDC):
                        _mm(nc, psk, wk_t[:, dc, h * 128:(h + 1) * 128],
                            xt_t[:, dc, :], dc == 0, dc == NDC - 1)
                    nc.vector.tensor_copy(
                        kt[:, h, sc * SC:(sc + 1) * SC], psk)
                for sb in range(SC // 128):
                    kb = sc * (SC // 128) + sb
                    psv = qkvp.tile([128, DHL], F32, tag="qkv")
                    for dc in range(NDC):
                        _mm(nc, psv, xt_t[:, dc, sb * 128:(sb + 1) * 128],
                            wv_t[:, dc, :], dc == 0, dc == NDC - 1)
                    nc.vector.tensor_copy(vcat[:, kb, 0, 0:128], psv[:, 0:128])
                    nc.vector.tensor_copy(vcat[:, kb, 1, 0:128],
                                          psv[:, 128:256])



                # ---- causal attention for q-chunk qi = sc ----
                qi = sc
                last_qi = (b == B - 1 and sc == NSC - 1)
                nkb = 4 * qi + 4
                # deferred[0:8] are transpose groups (run before first PV so
                # the acc banks' previous readers are enqueued first); the
                # rest are outproj groups, spread over the kb iterations.
                inj_T = deferred[:8]
                inj_O = deferred[8:]
                deferred = []
                n_iters = 2 * nkb - 1
                n_O = len(inj_O)
                it = 0

                acc_t = [accp.tile([128, 390], F32, tag=f"acc{i}",
                                   name=f"acc{i}")
                         for i in range(3)]
                st = {}
                lastq_pending = []

                def make_T(h, qql, acc_l=acc_t, st_=st):
                    def g():
                        r = 2 * qql + h
                        acc = acc_l[r // 3]
                        off = (r % 3) * 130
                        rc = sm.tile([128, 1], F32, tag="rc", bufs=8,
                                     name="rc")
                        nc.vector.reciprocal(
                            rc, acc[:, off + 128:off + 129])
                        an_s = sm.tile([128, 128], BF16, tag="an", bufs=8,
                                       name="an_s")
                        nc.vector.tensor_scalar_mul(
                            an_s, acc[:, off:off + 128], rc)
                        pst = qkvp.tile([128, 128], BF16, tag="qkv",
                                        name="pst")
                        nc.tensor.transpose(pst, an_s, ident)
                        stt = sm.tile([128, 128], BF16, tag="st", bufs=20,
                                      name="stt")
                        nc.vector.tensor_copy(stt, pst)
                        st_[(h, qql)] = stt
                    return g

                otw = {}

                def make_O(b_, qq, qql, dk, st_=st, otw_=otw):
                    def g():
                        # qkv psum pool is idle during attention injection
                        po = qkvp.tile([128, 512], F32, tag="qkv", name="po")
                        _mm(nc, po, st_[(0, qql)],
                            wo_t[:, 0, dk * 512:(dk + 1) * 512], True, False)
                        _mm(nc, po, st_[(1, qql)],
                            wo_t[:, 1, dk * 512:(dk + 1) * 512], False, True)
                        if dk == 0:
                            otw_[qql] = sm.tile([128, D], BF16, tag="ot",
                                                bufs=4, name="ot")
                        ot = otw_[qql]
                        if cast_split[0] and dk % 2 == 1:
                            nc.scalar.activation(
                                out=ot[:, dk * 512:(dk + 1) * 512], in_=po,
                                func=AF.Copy)
                        else:
                            nc.vector.tensor_copy(
                                ot[:, dk * 512:(dk + 1) * 512], po)
                        if last_qi:
                            # tail: narrow per-dk DMAs start draining sooner
                            eng = [nc.sync, nc.scalar][dk % 2]
                            eng.dma_start(
                                out=out[b_, qq * 128:(qq + 1) * 128,
                                        dk * 512:(dk + 1) * 512],
                                in_=ot[:, dk * 512:(dk + 1) * 512])
                        elif dk == D // 512 - 1:
                            # one wide DMA per q-block: 4KB descriptors
                            nc.sync.dma_start(
                                out=out[b_, qq * 128:(qq + 1) * 128, :],
                                in_=ot)
                    return g

                for h in range(HL):
                    for kb in range(nkb):
                        dq = max(0, (kb - 4 * qi)) * 128
                        pss = scp.tile([128, SC], F32, tag="sc")
                        _mm(nc, pss[:, dq:SC],
                            kt[:, h, kb * 128:(kb + 1) * 128],
                            qt[:, h, qi * SC + dq:(qi + 1) * SC], True, True)
                        ex = exp_.tile([128, SC], BF16, tag="ex")
                        nc.scalar.activation(out=ex[:, dq:SC],
                                             in_=pss[:, dq:SC], func=AF.Exp)
                        if kb >= 4 * qi:
                            nc.gpsimd.tensor_mul(
                                ex[:, dq:dq + 128], ex[:, dq:dq + 128], mask)
                        cast_split[0] = (qi <= 1)
                        if it == 0:
                            for g in inj_T:
                                g()
                        else:
                            lo = ((it - 1) * n_O) // n_iters
                            hi = (it * n_O) // n_iters
                            for g in inj_O[lo:hi]:
                                g()
                        it += 1
                        for qql in range(4):
                            qq = 4 * qi + qql
                            if kb <= qq:
                                # first program-order writer per bank (h is
                                # the outer loop): regions 0, 4, 6
                                r = 2 * qql + h
                                bk, off = r // 3, (r % 3) * 130
                                _mm(nc, acc_t[bk][:, off:off + 129],
                                    ex[:, qql * 128:(qql + 1) * 128],
                                    vcat[:, kb, h, :],
                                    kb == 0 and r in (0, 4, 6), kb == qq,
                                    skip_group_check=(r not in (0, 4, 6)))
                        if last_qi and h == HL - 1 and kb >= 4 * qi:
                            # drain as soon as each accumulator completes
                            cast_split[0] = True
                            qql = kb - 4 * qi
                            make_T(0, qql)()
                            make_T(1, qql)()
                            for dk in range(D // 512):
                                make_O(b, 4 * qi + qql, qql, dk)()

                for g in lastq_pending:
                    g()
                if not last_qi:
                    for qql in range(4):
                        for h in range(HL):
                            deferred.append(make_T(h, qql))
                    for qql in range(4):
                        for dk in range(D // 512):
                            deferred.append(make_O(b, 4 * qi + qql, qql, dk))

    nc.compile()
    return nc


def _get_nc():
    global _cached_nc
    if _cached_nc is None:
        _cached_nc = build_nc()
    return _cached_nc


def make_in_maps(X, Wq, bq, Wk, bk, Wv, bv, Wo, bo):
    import ml_dtypes
    bf16 = ml_dtypes.bfloat16
    X = np.asarray(X, dtype=np.float32)
    scale = np.float32(1.0 / np.sqrt(DH))
    # xt4[b, sc, j, p, i, s] = X^T[b, (4j+i)*128+p, sc*SC+s]
    XT = X.transpose(0, 2, 1)  # [B, D, S]
    xt4 = np.ascontiguousarray(
        XT.reshape(B, 4, 4, 128, NSC, SC).transpose(0, 4, 1, 3, 2, 5)
    ).astype(bf16)
    mi = np.concatenate([
        (np.arange(128)[None, :] >= np.arange(128)[:, None]),
        np.eye(128, dtype=bool)], axis=1).astype(bf16)
    bqt_pad = np.zeros((128, 16), np.float32)

    def wshard(W, hs, s=None):
        # [D, DHL] -> [128, NDC, DHL] with partition-major contiguous rows
        Ws = np.asarray(W, np.float32)[:, hs]
        if s is not None:
            Ws = Ws * s
        return np.ascontiguousarray(
            Ws.reshape(NDC, 128, DHL).transpose(1, 0, 2)).astype(bf16)

    in_maps = []
    for c in range(N_CORES):
        hs = slice(c * DHL, (c + 1) * DHL)
        bqt = bqt_pad.copy()
        bqt[:, 0:HL] = (np.asarray(bq, np.float32)[hs] * scale
                        ).reshape(HL, 128).T
        in_maps.append({
            "xt4": xt4,
            "wq": wshard(Wq, hs, scale),
            "wk": wshard(Wk, hs),
            "wv": wshard(Wv, hs),
            "bqt": bqt,
            "wo": np.ascontiguousarray(
                np.asarray(Wo, np.float32)[hs, :].reshape(HL, 128, D)
            ).astype(bf16),
            "mi": mi,
        })
    return in_maps


def kernel(X, Wq, bq, Wk, bk, Wv, bv, Wo, bo, _trace=False):
    nc = _get_nc()
    in_maps = make_in_maps(X, Wq, bq, Wk, bk, Wv, bv, Wo, bo)
    res = run_bass_kernel_spmd(nc, in_maps, list(range(N_CORES)), trace=_trace)
    acc = res.results[0]["out"].astype(np.float64)
    for c in range(1, N_CORES):
        acc += res.results[c]["out"].astype(np.float64)
    # bv commutes through softmax: sum_k w_k (v_k + bv) = (sum_k w_k v_k) + bv,
    # so the V bias contributes bv @ Wo, folded here with bo.
    acc += np.asarray(bo, np.float64) + (
        np.asarray(bv, np.float64) @ np.asarray(Wo, np.float64))
    out = acc.astype(np.float32)
    if _trace:
        return out, res
    return out


# revision 60
# speedup vs baseline: 1.0025x; 1.0025x over previous
"""Trainium2 Bass kernel for multi-head causal self-attention.

Tensor-parallel over 8 NeuronCores: each core owns 2 of the 16 heads.
Per core (SPMD, identical program, different weight shards), fused
chunk pipeline per batch: for each 512-row s-chunk, QKV projections
for that chunk, then causal attention for that q-chunk over all key
blocks produced so far.

All matmuls run in bf16 (fp32 PSUM accumulation). Scores kept
transposed [k, q]; softmax denominator via a ones column appended to
each head's V block ([V_h | 1], 129-wide PV matmuls). The exp on the
scalar engine is the attention inner-loop bottleneck, so the
transpose + output-projection work of q-chunk qi is deferred and
injected into qi+1's score/PV loop to fill the tensor-engine bubbles.

The 8 attention accumulators (2 heads x 4 q-blocks, 129 fp32 cols
each) are packed 3-per-PSUM-bank ([128,390], region r = 2*qql + h at
col (r%3)*130): only the first program-order matmul into each bank
uses start=True (clearing the whole bank's has_written bits); later
regions' first matmuls use start=False and overwrite via the
per-element has_written semantics.

Host: shards weights, pre-transposes X to bf16, sums 8 bf16 partials,
folds in bo + bv @ Wo (bv commutes through softmax; bk cancels).
"""
import numpy as np
from contextlib import ExitStack

import concourse.bass as bass
import concourse.tile as tile
from concourse import bacc, mybir
from concourse.bass_utils import run_bass_kernel_spmd

# Problem shape (hardcoded per contract)
B, S, D = 2, 2048, 2048
H, DH = 16, 128
N_CORES = 8
HL = H // N_CORES          # heads per core = 2
DHL = HL * DH              # 256
SC = 512                   # s-chunk width
NSC = S // SC              # 4 chunks per batch
NKB = S // 128             # 16 key blocks per batch
NDC = D // 128             # 16 contraction blocks

F32 = mybir.dt.float32
BF16 = mybir.dt.bfloat16
AF = mybir.ActivationFunctionType

_cached_nc = None


def _mm(nc, out, lhsT, rhs, start, stop, skip_group_check=False):
    nc.tensor.matmul(out, lhsT, rhs, start=start, stop=stop,
                     skip_group_check=skip_group_check)


def build_nc():
    nc = bacc.Bacc("TRN2", target_bir_lowering=False, debug=False,
                   num_devices=N_CORES)

    # all inputs pre-laid-out host-side as [128 partitions, ...contiguous]
    # so every DMA moves 4-8KB contiguous rows per partition (descriptor-
    # rate-bound otherwise: 1KB rows measured ~45 GB/s per queue)
    xt4 = nc.dram_tensor("xt4", [B, NSC, 4, 128, 4, SC], BF16,
                         kind="ExternalInput").ap()
    wq = nc.dram_tensor("wq", [128, NDC, DHL], BF16, kind="ExternalInput").ap()
    wk = nc.dram_tensor("wk", [128, NDC, DHL], BF16, kind="ExternalInput").ap()
    wv = nc.dram_tensor("wv", [128, NDC, DHL], BF16, kind="ExternalInput").ap()
    bqt_d = nc.dram_tensor("bqt", [128, 16], F32, kind="ExternalInput").ap()
    wo = nc.dram_tensor("wo", [HL, 128, D], BF16, kind="ExternalInput").ap()
    mi_d = nc.dram_tensor("mi", [128, 256], BF16, kind="ExternalInput").ap()
    out = nc.dram_tensor("out", [B, S, D], BF16, kind="ExternalOutput").ap()

    with tile.TileContext(nc) as tc, ExitStack() as ctx:
        pp = ctx.enter_context(tc.tile_pool(name="persist", bufs=1))

        wq_t = pp.tile([128, NDC, DHL], BF16)
        wk_t = pp.tile([128, NDC, DHL], BF16)
        wv_t = pp.tile([128, NDC, DHL], BF16)
        wo_t = pp.tile([128, HL, D], BF16)
        bqt = pp.tile([128, 16], F32)
        mi = pp.tile([128, 256], BF16)
        mask = mi[:, 0:128]
        ident = mi[:, 128:256]

        # need-ordered startup on the two hardware DMA queues (sync/scalar);
        # gpsimd's software queue has ~5us pickup latency, use it mid-kernel

        # double-buffered across batches
        qt_b = [pp.tile([128, HL, S], BF16, name=f"qt{i}") for i in range(2)]
        kt_b = [pp.tile([128, HL, S], BF16, name=f"kt{i}") for i in range(2)]
        vcat_b = [pp.tile([128, NKB, HL, 129], BF16, name=f"vc{i}")
                  for i in range(2)]

        xp = ctx.enter_context(tc.tile_pool(name="xtp", bufs=3))
        qkvp = ctx.enter_context(
            tc.tile_pool(name="qkvp", bufs=3, space="PSUM"))
        scp = ctx.enter_context(tc.tile_pool(name="scp", bufs=2, space="PSUM"))
        accp = ctx.enter_context(tc.tile_pool(name="accp", bufs=1, space="PSUM"))
        exp_ = ctx.enter_context(tc.tile_pool(name="exp", bufs=10))
        sm = ctx.enter_context(tc.tile_pool(name="sm", bufs=2))

        deferred = []  # closures: transpose + outproj work of previous qi
        cast_split = [False]  # route half the ot casts to scalar when set

        for b in range(B):
            qt, kt, vcat = qt_b[b % 2], kt_b[b % 2], vcat_b[b % 2]
            nc.gpsimd.memset(vcat[:, :, :, 128:129], 1.0)

            for sc in range(NSC):
                # ---- QKV projections for this chunk ----
                xt_t = xp.tile([128, NDC, SC], BF16, tag="xt")
                if b == 0 and sc == 0:
                    # startup burst: fine slices interleaved by need-time
                    # across both hw queues (aggregate DMA ~370GB/s shared)
                    nc.scalar.dma_start(out=wq_t[:, 0:4, :], in_=wq[:, 0:4, :])
                    nc.sync.dma_start(out=xt_t[:, 0:4, :], in_=xt4[0, 0, 0])
                    nc.scalar.dma_start(out=wq_t[:, 4:8, :], in_=wq[:, 4:8, :])
                    nc.sync.dma_start(out=xt_t[:, 4:8, :], in_=xt4[0, 0, 1])
                    nc.scalar.dma_start(out=wq_t[:, 8:NDC, :],
                                        in_=wq[:, 8:NDC, :])
                    nc.sync.dma_start(out=xt_t[:, 8:12, :], in_=xt4[0, 0, 2])
                    nc.scalar.dma_start(out=wk_t[:, 0:8, :], in_=wk[:, 0:8, :])
                    nc.sync.dma_start(out=xt_t[:, 12:16, :],
                                      in_=xt4[0, 0, 3])
                    nc.sync.dma_start(out=bqt, in_=bqt_d)
                    nc.scalar.dma_start(out=wk_t[:, 8:NDC, :],
                                        in_=wk[:, 8:NDC, :])
                    nc.sync.dma_start(out=wv_t, in_=wv)
                    nc.scalar.dma_start(out=mi, in_=mi_d)
                elif b == 0 and sc == 1:
                    # scalar engine reaches these after chunk0's activations,
                    # keeping the startup burst clear of non-critical bytes
                    for part in range(4):
                        nc.scalar.dma_start(
                            out=xt_t[:, 4 * part:4 * part + 4, :],
                            in_=xt4[b, sc, part])
                    nc.scalar.dma_start(out=wo_t[:, 0, :], in_=wo[0])
                    nc.scalar.dma_start(out=wo_t[:, 1, :], in_=wo[1])
                else:
                    for part in range(4):
                        eng = nc.gpsimd if part % 2 == 0 else nc.sync
                        eng.dma_start(
                            out=xt_t[:, 4 * part:4 * part + 4, :],
                            in_=xt4[b, sc, part])
                for h in range(HL):
                    psq = qkvp.tile([128, SC], F32, tag="qkv")
                    for dc in range(NDC):
                        _mm(nc, psq, wq_t[:, dc, h * 128:(h + 1) * 128],
                            xt_t[:, dc, :], dc == 0, dc == NDC - 1)
                    nc.scalar.activation(
                        out=qt[:, h, sc * SC:(sc + 1) * SC], in_=psq,
                        func=AF.Identity, bias=bqt[:, h:h + 1], scale=1.0)
                    psk = qkvp.tile([128, SC], F32, tag="qkv")
                    for dc in range(NDC):
                        _mm(nc, psk, wk_t[:, dc, h * 128:(h + 1) * 128],
                            xt_t[:, dc, :], dc == 0, dc == NDC - 1)
                    nc.vector.tensor_copy(
                        kt[:, h, sc * SC:(sc + 1) * SC], psk)
                for sb in range(SC // 128):
                    kb = sc * (SC // 128) + sb
                    psv = qkvp.tile([128, DHL], F32, tag="qkv")
                    for dc in range(NDC):
                        _mm(nc, psv, xt_t[:, dc, sb * 128:(sb + 1) * 128],
                            wv_t[:, dc, :], dc == 0, dc == NDC - 1)
                    nc.vector.tensor_copy(vcat[:, kb, 0, 0:128], psv[:, 0:128])
                    nc.vector.tensor_copy(vcat[:, kb, 1, 0:128],
                                          psv[:, 128:256])



                # ---- causal attention for q-chunk qi = sc ----
                qi = sc
                last_qi = (b == B - 1 and sc == NSC - 1)
                nkb = 4 * qi + 4
                # deferred[0:8] are transpose groups (run before first PV so
                # the acc banks' previous readers are enqueued first); the
                # rest are outproj groups, spread over the kb iterations.
                inj_T = deferred[:8]
                inj_O = deferred[8:]
                deferred = []
                n_iters = 2 * nkb - 1
                n_O = len(inj_O)
                it = 0

                acc_t = [accp.tile([128, 390], F32, tag=f"acc{i}",
                                   name=f"acc{i}")
                         for i in range(3)]
                st = {}
                lastq_pending = []

                def make_T(h, qql, acc_l=acc_t, st_=st):
                    def g():
                        r = 2 * qql + h
                        acc = acc_l[r // 3]
                        off = (r % 3) * 130
                        rc = sm.tile([128, 1], F32, tag="rc", bufs=8,
                                     name="rc")
                        nc.vector.reciprocal(
                            rc, acc[:, off + 128:off + 129])
                        an_s = sm.tile([128, 128], BF16, tag="an", bufs=8,
                                       name="an_s")
                        nc.vector.tensor_scalar_mul(
                            an_s, acc[:, off:off + 128], rc)
                        pst = qkvp.tile([128, 128], BF16, tag="qkv",
                                        name="pst")
                        nc.tensor.transpose(pst, an_s, ident)
                        stt = sm.tile([128, 128], BF16, tag="st", bufs=20,
                                      name="stt")
                        nc.vector.tensor_copy(stt, pst)
                        st_[(h, qql)] = stt
                    return g

                otw = {}

                def make_O(b_, qq, qql, dk, st_=st, otw_=otw):
                    def g():
                        # qkv psum pool is idle during attention injection
                        po = qkvp.tile([128, 512], F32, tag="qkv", name="po")
                        _mm(nc, po, st_[(0, qql)],
                            wo_t[:, 0, dk * 512:(dk + 1) * 512], True, False)
                        _mm(nc, po, st_[(1, qql)],
                            wo_t[:, 1, dk * 512:(dk + 1) * 512], False, True)
                        if dk == 0:
                            otw_[qql] = sm.tile([128, D], BF16, tag="ot",
                                                bufs=4, name="ot")
                        ot = otw_[qql]
                        if cast_split[0] and dk % 2 == 1:
                            nc.scalar.activation(
                                out=ot[:, dk * 512:(dk + 1) * 512], in_=po,
                                func=AF.Copy)
                        else:
                            nc.vector.tensor_copy(
                                ot[:, dk * 512:(dk + 1) * 512], po)
                        if last_qi:
                            # tail: narrow per-dk DMAs start draining sooner
                            eng = [nc.sync, nc.scalar][dk % 2]
                            eng.dma_start(
                                out=out[b_, qq * 128:(qq + 1) * 128,
                                        dk * 512:(dk + 1) * 512],
                                in_=ot[:, dk * 512:(dk + 1) * 512])
                        elif dk == D // 512 - 1:
                            # one wide DMA per q-block: 4KB descriptors
                            nc.sync.dma_start(
                                out=out[b_, qq * 128:(qq + 1) * 128, :],
                                in_=ot)
                    return g

                for h in range(HL):
                    for kb in range(nkb):
                        dq = max(0, (kb - 4 * qi)) * 128
                        pss = scp.tile([128, SC], F32, tag="sc")
                        _mm(nc, pss[:, dq:SC],
                            kt[:, h, kb * 128:(kb + 1) * 128],
                            qt[:, h, qi * SC + dq:(qi + 1) * SC], True, True)
                        ex = exp_.tile([128, SC], BF16, tag="ex")
                        nc.scalar.activation(out=ex[:, dq:SC],
                                             in_=pss[:, dq:SC], func=AF.Exp)
                        if kb >= 4 * qi:
                            nc.gpsimd.tensor_mul(
                                ex[:, dq:dq + 128], ex[:, dq:dq + 128], mask)
                        cast_split[0] = (qi <= 1)
                        if it == 0:
                            for g in inj_T:
                                g()
                        else:
                            lo = ((it - 1) * n_O) // n_iters
                            hi = (it * n_O) // n_iters
                            for g in inj_O[lo:hi]:
                                g()
                        it += 1
                        for qql in range(4):
                            qq = 4 * qi + qql
                            if kb <= qq:
                                # first program-order writer per bank (h is
                                # the outer loop): regions 0, 4, 6
                                r = 2 * qql + h
                                bk, off = r // 3, (r % 3) * 130
                                _mm(nc, acc_t[bk][:, off:off + 129],
                                    ex[:, qql * 128:(qql + 1) * 128],
                                    vcat[:, kb, h, :],
                                    kb == 0 and r in (0, 4, 6), kb == qq,
                                    skip_group_check=(r not in (0, 4, 6)))
                        if last_qi and h == HL - 1 and kb >= 4 * qi:
                            # drain as soon as each accumulator completes
                            cast_split[0] = True
                            qql = kb - 4 * qi
                            make_T(0, qql)()
                            make_T(1, qql)()
                            for dk in range(D // 512):
                                make_O(b, 4 * qi + qql, qql, dk)()

                for g in lastq_pending:
                    g()
                if not last_qi:
                    for qql in range(4):
                        for h in range(HL):
                            deferred.append(make_T(h, qql))
                    for qql in range(4):
                        for dk in range(D // 512):
                            deferred.append(make_O(b, 4 * qi + qql, qql, dk))

    nc.compile()
    return nc


def _get_nc():
    global _cached_nc
    if _cached_nc is None:
        _cached_nc = build_nc()
    return _cached_nc


def make_in_maps(X, Wq, bq, Wk, bk, Wv, bv, Wo, bo):
    import ml_dtypes
    bf16 = ml_dtypes.bfloat16
    X = np.asarray(X, dtype=np.float32)
    scale = np.float32(1.0 / np.sqrt(DH))
    # xt4[b, sc, j, p, i, s] = X^T[b, (4j+i)*128+p, sc*SC+s]
    XT = X.transpose(0, 2, 1)  # [B, D, S]
    xt4 = np.ascontiguousarray(
        XT.reshape(B, 4, 4, 128, NSC, SC).transpose(0, 4, 1, 3, 2, 5)
    ).astype(bf16)
    mi = np.concatenate([
        (np.arange(128)[None, :] >= np.arange(128)[:, None]),
        np.eye(128, dtype=bool)], axis=1).astype(bf16)
    bqt_pad = np.zeros((128, 16), np.float32)

    def wshard(W, hs, s=None):
        # [D, DHL] -> [128, NDC, DHL] with partition-major contiguous rows
        Ws = np.asarray(W, np.float32)[:, hs]
        if s is not None:
            Ws = Ws * s
        return np.ascontiguousarray(
            Ws.reshape(NDC, 128, DHL).transpose(1, 0, 2)).astype(bf16)

    in_maps = []
    for c in range(N_CORES):
        hs = slice(c * DHL, (c + 1) * DHL)
        bqt = bqt_pad.copy()
        bqt[:, 0:HL] = (np.asarray(bq, np.float32)[hs] * scale
                        ).reshape(HL, 128).T
        in_maps.append({
            "xt4": xt4,
            "wq": wshard(Wq, hs, scale),
            "wk": wshard(Wk, hs),
            "wv": wshard(Wv, hs),
            "bqt": bqt,
            "wo": np.ascontiguousarray(
                np.asarray(Wo, np.float32)[hs, :].reshape(HL, 128, D)
            ).astype(bf16),
            "mi": mi,
        })
    return in_maps


def kernel(X, Wq, bq, Wk, bk, Wv, bv, Wo, bo, _trace=False):
    nc = _get_nc()
    in_maps = make_in_maps(X, Wq, bq, Wk, bk, Wv, bv, Wo, bo)
    res = run_bass_kernel_spmd(nc, in_maps, list(range(N_CORES)), trace=_trace)
    acc = res.results[0]["out"].astype(np.float64)
    for c in range(1, N_CORES):
        acc += res.results[c]["out"].astype(np.float64)
    # bv commutes through softmax: sum_k w_k (v_k + bv) = (sum_k w_k v_k) + bv,
    # so the V bias contributes bv @ Wo, folded here with bo.
    acc += np.asarray(bo, np.float64) + (
        np.asarray(bv, np.float64) @ np.asarray(Wo, np.float64))
    out = acc.astype(np.float32)
    if _trace:
        return out, res
    return out
